# Initial kernel scaffold
#
"""DiffLogic network TRN2 kernel: 3 logic layers [B=256, W=64000] + GroupSum.

Sharding: pure data-parallel over batch across 8 cores (B=32/core), no
inter-core communication. Per core: activations h stored in DRAM as
[64000, 128] bf16 rows (256B stride, first 32 cols real = 64B payload).
Gathers a=h[idx_a], b=h[idx_b] via SWDGE dma_gather with elem_size=32
(64B descriptors — 7ns floor instead of 22.8ns for 256B) and signed
int16 indices (base at row 32000). h writes go over HWDGE (SP engine)
to keep the Pool engine free for gather descriptor generation.
Gate = (c1+c3*b)*a + (c0+c2*b) computed on DVE (6 tensor-tensor ops,
kept in the 2x perf mode by storing coefficients as stride-1 pairs);
coefs = softmax(w)@G via ACT exp + PE matmuls contracting the 16-gate
axis on the partition dim. GroupSum via PE one-hot matmul.
"""
import numpy as np
import ml_dtypes

import concourse.bass as bass
import concourse.tile as tile
import concourse.bacc as bacc
import concourse.mybir as mybir
from concourse import ap_utils
from concourse.bass_utils import run_bass_kernel_spmd
from concourse.library_config import mlp

W = 64000
BATCH = 256
NCORES = 8
BC = BATCH // NCORES        # 32 batch rows per core
IN_DIM = 1024
K = 10
TAU = 30.0
NSLOT = W // 128            # 500
E = 128                     # elements per h row (256B stride); [:32] real
CHUNK_SLOTS = 64            # neurons per chunk = 64*128 = 8192
GPN = 8192                  # idxs per dma_gather instruction (needs single_packet=False)
H_BASE = 32000              # gather base row (signed int16 rebase)

GATE_COEF = np.array([
    [0., 0., 0., 0.], [0., 0., 0., 1.], [0., 1., 0., -1.], [0., 1., 0., 0.],
    [0., 0., 1., -1.], [0., 0., 1., 0.], [0., 1., 1., -2.], [0., 1., 1., -1.],
    [1., -1., -1., 1.], [1., -1., -1., 2.], [1., 0., -1., 0.], [1., 0., -1., 1.],
    [1., -1., 0., 0.], [1., -1., 0., 1.], [1., 0., 0., -1.], [1., 0., 0., 0.],
], dtype=np.float32)  # [16, 4]

BF16 = mybir.dt.bfloat16
F32 = mybir.dt.float32
I16 = mybir.dt.int16
IDX_COLS = W // 16  # wrapped idx tensor cols per list

_NC_CACHE = {}


def dma_gather_small(gp, out_ap, in_ap, idxs_ap, num_idxs, elem_size, elem_step):
    """dma_gather without the 256B elem-size assert (non-transpose, DRAM src).
    The 256B restriction only applies to transpose mode in the ucode; the
    source row stride (elem_step) must still be a multiple of 256B."""
    assert idxs_ap.dtype == mybir.dt.int16
    assert in_ap.dtype == out_ap.dtype
    assert ap_utils.ap_is_contiguous(out_ap.ap[1:])
    assert ap_utils.ap_is_contiguous(idxs_ap.ap[1:])
    assert in_ap.ap[-1][1] == out_ap.ap[-1][1] == elem_size
    assert in_ap.ap[0][0] == elem_step
    stride_bytes = elem_step * mybir.dt.size(in_ap.dtype)
    stride_256 = stride_bytes // 256
    assert stride_256 * 256 == stride_bytes and stride_256 < 256
    _in_ap = gp.lower_ap_dma(in_ap, for_custom_bir_dma=True)
    _idxs_ap = gp.lower_ap(idxs_ap)
    _out_ap = gp.lower_ap(out_ap)
    return gp.add_instruction(
        mybir.InstDMAGatherAnt(
            name=gp.bass.get_next_instruction_name(),
            ins=[*_in_ap, _idxs_ap, gp.lower_val_access(gp.to_reg(num_idxs))],
            outs=[_out_ap],
            transpose=False,
            num_idxs=num_idxs,
            elem_size=elem_size,
            stride_bytes_256=stride_256,
            gen_mode=0,
            single_packet=False,
            queue_num=0,
            sbuf_tokens_per_rank=0,
            sbuf_free_dim_per_rank=0,
            sbuf_free_dim_pad_per_rank=0,
            sbuf_byte_offset=0,
        ))


def _chunks():
    """Yield (slot0, nslots) chunks over the 500 slots."""
    s = 0
    while s < NSLOT:
        n = min(CHUNK_SLOTS, NSLOT - s)
        yield s, n
        s += n


def _gathers(nslots):
    """Split a chunk of nslots*128 idxs into per-instruction counts."""
    n = nslots * 128
    out = []
    while n > 0:
        g = min(GPN, n)
        out.append(g)
        n -= g
    return out


S1 = 512        # layer-1 grid slots: 8 affine x-columns + spill tail
NPOS1 = S1 * 128


def build_nc(bounds=None, l0meta=None):
    """l0meta: (cols, tail_ns) — cols = [(s0, ns)] for the 8 affine
    x-columns of layer 1 (a read via broadcast from xs, no a-gather),
    tail_ns = slots of the spill chunk (both lists gathered)."""
    if "nc" in _NC_CACHE:
        return _NC_CACHE["nc"]
    nc = bacc.Bacc("TRN2", target_bir_lowering=False, debug=False,
                   enable_asserts=False, num_devices=NCORES)

    NQ1 = S1 // 8          # 64 PE-coef matmuls for layer 1
    NQ = (NSLOT + 7) // 8  # 63 for layers 2-3

    xT = nc.dram_tensor("xT", [IN_DIM, E], BF16, kind="ExternalInput")
    xs = nc.dram_tensor("xs", [128, 8, 32], BF16, kind="ExternalInput")
    # wf2[(k,m), q, K] = w[neuron(p=K, s=8q+m), k]  (PE-coef layout)
    wf = [nc.dram_tensor(f"wf{l}", [128, NQ1 if l == 0 else NQ, 128], BF16,
                         kind="ExternalInput") for l in range(3)]
    ia = [nc.dram_tensor("ia0", [128, 256], I16, kind="ExternalInput")] + [
        nc.dram_tensor(f"ia{l}", [128, IDX_COLS], I16, kind="ExternalInput")
        for l in (1, 2)]
    ib = [nc.dram_tensor("ib0", [128, NPOS1 // 16], I16, kind="ExternalInput")] + [
        nc.dram_tensor(f"ib{l}", [128, IDX_COLS], I16, kind="ExternalInput")
        for l in (1, 2)]
    g10 = nc.dram_tensor("g10", [128, NSLOT, K], BF16, kind="ExternalInput")
    # ghat[(k,m), 5j+mm] = G[k, j] * (m == mm); j=4 row is ones (softmax denom)
    ghat = nc.dram_tensor("ghat", [128, 40], BF16, kind="ExternalInput")
    h_dram = [nc.dram_tensor("h0", [NPOS1, E], BF16, kind="Internal"),
              nc.dram_tensor("h1", [W, E], BF16, kind="Internal")]
    out_dram = nc.dram_tensor("out", [K, BC], F32, kind="ExternalOutput")

    with tile.TileContext(nc) as tc:
        with (
            tc.tile_pool(name="persist", bufs=1) as persist,
            tc.tile_pool(name="idxp", bufs=2) as idxp,
            tc.tile_pool(name="coef", bufs=1) as coefp,
            tc.tile_pool(name="gath", bufs=3) as gath,
            tc.tile_pool(name="temps", bufs=4) as temps,
            tc.tile_pool(name="psum", bufs=1, space="PSUM") as psump,
        ):
            nc.gpsimd.load_library(mlp)

            # persistent loads
            ghat_sb = persist.tile([128, 40], BF16, tag="ghat")
            nc.sync.dma_start(ghat_sb[:], ghat[:])
            g10_sb = persist.tile([128, NSLOT, K], BF16, tag="g10")
            nc.sync.dma_start(g10_sb[:], g10[:])

            psum_out = psump.tile([K, BC], F32, tag="acc")
            n_mm = NSLOT  # total groupsum matmuls
            mm_i = 0

            xs_sb = persist.tile([128, 8, 32], BF16, tag="xs")
            nc.sync.dma_start(xs_sb[:], xs[:])

            for l in range(3):
                NQl = NQ1 if l == 0 else NQ
                Sl = S1 if l == 0 else NSLOT
                if l == 0:
                    ia_sb = idxp.tile([128, 256], I16, tag="iat", name="ia_sb")
                    ib_sb = idxp.tile([128, NPOS1 // 16], I16, tag="ib0t",
                                      name="ib_sb")
                else:
                    ia_sb = idxp.tile([128, IDX_COLS], I16, tag="ia", name="ia_sb")
                    ib_sb = idxp.tile([128, IDX_COLS], I16, tag="ib", name="ib_sb")
                nc.sync.dma_start(ia_sb[:], ia[l][:])
                nc.sync.dma_start(ib_sb[:], ib[l][:])

                # ---- coefficient prep: coef = softmax(w) @ GATE_COEF via PE ----
                # craw[j, n] for neurons n=(K, s=8q+m): matmul q contracts over
                # the (gate k, m) partition dim: out[K, j*8+mm] = sum_p
                # e_fold[p, K] * ghat[p, j*8+mm].
                wf_sb = coefp.tile([128, NQ1, 128], BF16, tag="wf", name="wf_sb")
                nc.sync.dma_start(wf_sb[:, :NQl, :], wf[l][:])
                e_sb = coefp.tile([128, NQ1, 128], BF16, tag="e", name="e_sb")
                nc.scalar.activation(e_sb[:, :NQl, :], wf_sb[:, :NQl, :],
                                     mybir.ActivationFunctionType.Exp)
                c_sb = coefp.tile([128, NQ1, 40], BF16, tag="csb", name="c_sb")
                QG = 12  # matmuls per PSUM bank group
                q0 = 0
                while q0 < NQl:
                    nq = min(QG, NQl - q0)
                    cps = psump.tile([128, QG, 40], F32, tag="cps", name="cps")
                    for qi in range(nq):
                        nc.tensor.matmul(cps[:, qi, :],
                                         lhsT=e_sb[:, q0 + qi, :],
                                         rhs=ghat_sb[:],
                                         start=True, stop=True)
                    nc.scalar.mul(c_sb[:, q0:q0 + nq, :], cps[:, :nq, :], 1.0)
                    q0 += nq
                rinv = coefp.tile([128, NQ1, 8], F32, tag="rinv", name="rinv")
                nc.vector.reciprocal(out=rinv[:, :NQl, :], in_=c_sb[:, :NQl, 32:40])
                # cjd[j][p, s, 0:2] = coef_j[p, s] twice (stride-1 pair so the
                # gate ops' broadcast operand keeps the DVE 2x perf mode)
                cjd = [coefp.tile([128, NQ1 * 8, 2], BF16, tag=f"c{j}",
                                  name=f"cjd{j}") for j in range(4)]
                for j in range(4):
                    dst = (cjd[j][:, :NQl * 8, :]
                           .rearrange("p (q m) t -> p q m t", m=8))
                    cj_s = (c_sb[:, :NQl, j * 8:(j + 1) * 8].unsqueeze(-1)
                            .to_broadcast([128, NQl, 8, 2]))
                    ri_s = (rinv[:, :NQl, :].unsqueeze(-1)
                            .to_broadcast([128, NQl, 8, 2]))
                    nc.vector.tensor_mul(dst, cj_s, ri_s)

                # ---- gather + gate over chunks ----
                def srcs(gi):
                    """Per-instruction source windows: base from the per-
                    instruction int16 rebase, upper bound = max row touched
                    (consumers are sorted by source row, so early gathers
                    depend only on early h writes and can overlap the
                    previous layer's tail)."""
                    if l == 0:
                        return xT[:, :32], xT[:, :32]
                    assert bounds is not None, "layers 2-3 need rebase info"
                    ba, bda, bb, bdb = bounds[l - 1]
                    return (h_dram[l - 1][ba[gi]:bda[gi], :32],
                            h_dram[l - 1][bb[gi]:bdb[gi], :32])

                if l == 0:
                    cols, tail_ns = l0meta
                    chunks = [(s0, ns, i) for i, (s0, ns) in enumerate(cols)]
                    chunks.append((S1 - tail_ns, tail_ns, None))
                else:
                    chunks = [(s0, ns, None) for s0, ns in _chunks()]

                gi = 0
                acol = 0  # a-idx cols consumed (layer-0 tail only)
                for s0, ns, xcol in chunks:
                    a_t = gath.tile([128, CHUNK_SLOTS, 32], BF16, tag="a")
                    b_t = gath.tile([128, CHUNK_SLOTS, 32], BF16, tag="b")
                    col = s0 * 8  # b-idx cols consumed so far (128/16 per slot)
                    slot = 0
                    for n in _gathers(ns):
                        ncols = n // 16
                        nslots_g = n // 128
                        src_a, src_b = srcs(gi)
                        gi += 1
                        if xcol is None:
                            dma_gather_small(
                                nc.gpsimd, a_t[:, slot:slot + nslots_g, :],
                                src_a, ia_sb[:, acol:acol + ncols], n, 32, E)
                            acol += ncols
                        dma_gather_small(
                            nc.gpsimd, b_t[:, slot:slot + nslots_g, :], src_b,
                            ib_sb[:, col:col + ncols], n, 32, E)
                        col += ncols
                        slot += nslots_g

                    # 4-dim views with stride-1 inner pairs keep DVE 2x mode
                    if xcol is None:
                        av = a_t[:, :ns, :].rearrange("p c (g t) -> p c g t", t=2)
                    else:
                        av = (xs_sb[:, xcol, :].rearrange("p (g t) -> p g t", t=2)
                              .unsqueeze(1).to_broadcast([128, ns, 16, 2]))
                    bv = b_t[:, :ns, :].rearrange("p c (g t) -> p c g t", t=2)

                    def cbc(j):
                        return (cjd[j][:, s0:s0 + ns, :].unsqueeze(2)
                                .to_broadcast([128, ns, 16, 2]))

                    # gate = (c1 + c3*b)*a + (c0 + c2*b): 6 DVE ops
                    m1 = temps.tile([128, CHUNK_SLOTS, 32], BF16, tag="m1")
                    m2 = temps.tile([128, CHUNK_SLOTS, 32], BF16, tag="m2")
                    m1v = m1[:, :ns, :].rearrange("p c (g t) -> p c g t", t=2)
                    m2v = m2[:, :ns, :].rearrange("p c (g t) -> p c g t", t=2)
                    nc.vector.tensor_mul(m1v, bv, cbc(3))
                    nc.vector.tensor_add(m1v, m1v, cbc(1))
                    nc.vector.tensor_mul(m1v, m1v, av)
                    nc.vector.tensor_mul(m2v, bv, cbc(2))
                    nc.vector.tensor_add(m2v, m2v, cbc(0))
                    nc.vector.tensor_add(m1v, m1v, m2v)

                    if l < 2:
                        # write rows (s0+c)*128+p of h_dram[l] (64B @ 256B stride)
                        hap = h_dram[l].ap()
                        dst = hap[s0 * 128: s0 * 128 + ns * 128, :32]
                        dst = dst.rearrange("(c p) e -> p c e", p=128)
                        nc.sync.dma_start(dst, m1[:, :ns, :])
                    else:
                        for c in range(ns):
                            nc.tensor.matmul(
                                psum_out[:],
                                lhsT=g10_sb[:, s0 + c, :],
                                rhs=m1[:, c, :],
                                start=(mm_i == 0),
                                stop=(mm_i == n_mm - 1),
                            )
                            mm_i += 1

            out_sb = persist.tile([K, BC], F32, tag="outsb")
            nc.scalar.mul(out_sb[:], psum_out[:], 1.0 / TAU)
            nc.sync.dma_start(out_dram[:], out_sb[:])

    nc.compile()
    _NC_CACHE["nc"] = nc
    return nc


def _wrap(idx):
    """Flat idx list [n] -> [128, n/16] int16 wrapped per 16 partitions,
    replicated to the 8 gpsimd cores."""
    n = idx.shape[0]
    arr = np.empty((128, n // 16), dtype=np.int16)
    blk = idx.reshape(n // 16, 16).T.astype(np.int16)
    for g in range(8):
        arr[g * 16:(g + 1) * 16, :] = blk
    return arr


def _fix_trailing(idx_a, idx_b, perm=None):
    """Ensure the last idx of every GPN-sublist is >= 0 for both lists
    (SWDGE trims trailing negatives). Returns permuted lists + perm."""
    perm = np.arange(W) if perm is None else perm.copy()
    a = idx_a.copy()
    b = idx_b.copy()
    pos = 0
    for s0, ns in _chunks():
        for n in _gathers(ns):
            last = pos + n - 1
            if a[last] < 0 or b[last] < 0:
                ok = np.nonzero((a[pos:last] >= 0) & (b[pos:last] >= 0))[0]
                j = pos + int(ok[-1])
                for arr in (a, b, perm):
                    arr[last], arr[j] = arr[j], arr[last]
            pos += n
    return a, b, perm


def _fold(x):
    """[W, ...] -> [128, NSLOT, ...] with row n=(c*128+p) at [p, c]."""
    return np.ascontiguousarray(
        x.reshape(NSLOT, 128, *x.shape[1:]).transpose(1, 0, *range(2, x.ndim + 1)))


def kernel(x, w1, w2, w3, idx_a1, idx_b1, idx_a2, idx_b2, idx_a3, idx_b3):
    x = np.asarray(x, dtype=np.float32)
    ws = [np.asarray(w, dtype=np.float32) for w in (w1, w2, w3)]
    ias = [np.asarray(i).astype(np.int64) for i in (idx_a1, idx_a2, idx_a3)]
    ibs = [np.asarray(i).astype(np.int64) for i in (idx_b1, idx_b2, idx_b3)]

    # ---- host-side index translation / layout prep (shared across cores) ----
    # Layer 1: assign x value of count-rank r to cell (p=r%128, col=r//128);
    # consumers sit in their a-source's cell (up to the column cap), so the
    # a-operand is an affine broadcast from xs and needs NO gather. Excess
    # consumers spill to a tail chunk where both lists are gathered.
    T_CAP = 64
    counts = np.bincount(ias[0], minlength=IN_DIM)
    order_v = np.argsort(-counts, kind="stable")
    Mcol = np.minimum(counts[order_v].reshape(8, 128).max(axis=1),
                      T_CAP).astype(np.int64)
    offs = np.concatenate([[0], np.cumsum(Mcol)])
    S_AFF = int(Mcol.sum())
    TAIL = S1 - S_AFF
    cols = [(int(offs[i]), int(Mcol[i])) for i in range(8)]
    ordc = np.argsort(ias[0], kind="stable")
    starts = np.concatenate([[0], np.cumsum(counts)])
    perm0 = np.full(NPOS1, -1, dtype=np.int64)
    tail_cons = []
    for r in range(IN_DIM):
        v = int(order_v[r])
        p, i = r % 128, r // 128
        cons = ordc[starts[v]:starts[v + 1]]
        k = min(len(cons), int(Mcol[i]))
        perm0[(offs[i] + np.arange(k)) * 128 + p] = cons[:k]
        tail_cons.extend(cons[k:].tolist())
    assert len(tail_cons) <= TAIL * 128
    perm0[S_AFF * 128 + np.arange(len(tail_cons))] = tail_cons
    mask0 = perm0 >= 0
    b1 = np.zeros(NPOS1, dtype=np.int64)
    b1[mask0] = ibs[0][perm0[mask0]]
    a_tail = np.zeros(TAIL * 128, dtype=np.int64)
    tmask = mask0[S_AFF * 128:]
    a_tail[tmask] = ias[0][perm0[S_AFF * 128:][tmask]]
    a_tail_pad = np.zeros(256 * 16, dtype=np.int64)
    a_tail_pad[:TAIL * 128] = a_tail

    perms = [perm0]
    lists = [None]
    bounds = []
    for l in (1, 2):
        pm = perms[l - 1]
        inv_prev = np.empty(W, dtype=np.int64)
        msk = pm >= 0
        inv_prev[pm[msk]] = np.nonzero(msk)[0]
        ra = inv_prev[ias[l]]
        rb = inv_prev[ibs[l]]
        # sort consumers by max source row: early gather instructions then
        # only touch early h rows, so (with per-instruction src-AP windows)
        # they can start before the previous layer finishes writing h
        order = np.argsort(np.maximum(ra, rb), kind="stable")
        ra2, rb2 = ra[order].copy(), rb[order].copy()
        # per-instruction int16 rebase: base = max(0, hi-32767) keeps every
        # idx in range; sorted sublists have narrow spans so most have no
        # negative idxs at all
        binfo = ([], [], [], [])  # base_a, bound_a, base_b, bound_b
        pos = 0
        for s0, ns in _chunks():
            for n in _gathers(ns):
                sl = slice(pos, pos + n)
                for arr, k in ((ra2, 0), (rb2, 2)):
                    hi = int(arr[sl].max())
                    base = max(0, hi - 32767)
                    arr[sl] -= base
                    binfo[k].append(base)
                    binfo[k + 1].append(hi + 1)
                pos += n
        ra3, rb3, perm = _fix_trailing(ra2, rb2, perm=order)
        perms.append(perm)
        lists.append((ra3, rb3))
        bounds.append(binfo)

    nc = build_nc(bounds, l0meta=(cols, TAIL))

    NQ = (NSLOT + 7) // 8
    NQ1 = S1 // 8

    def _wf_fold(wp, S, NQl):
        # wf2[k*8+m, q, K] = wp[(8q+m)*128 + K, k]
        wf2 = np.zeros((128, NQl, 128), dtype=np.float32)
        for m in range(8):
            s_ids = 8 * np.arange(NQl) + m
            valid = s_ids < S
            n = s_ids[valid][:, None] * 128 + np.arange(128)[None, :]
            vals = wp[n, :].transpose(2, 0, 1)    # [16, nq_v, 128]
            tmp = np.zeros((16, NQl, 128), dtype=np.float32)
            tmp[:, valid, :] = vals
            wf2[np.arange(16) * 8 + m] = tmp
        return wf2.astype(ml_dtypes.bfloat16)

    shared = {}
    shared["ia0"] = _wrap(a_tail_pad)
    shared["ib0"] = _wrap(b1)
    wp0 = np.zeros((NPOS1, 16), dtype=np.float32)
    wp0[mask0] = ws[0][perm0[mask0]]
    shared["wf0"] = _wf_fold(wp0, S1, NQ1)
    for l in (1, 2):
        a, b = lists[l]
        shared[f"ia{l}"] = _wrap(a)
        shared[f"ib{l}"] = _wrap(b)
        shared[f"wf{l}"] = _wf_fold(ws[l][perms[l]], NSLOT, NQ)

    group = perms[2] // (W // K)          # group id of neuron at list pos j
    g10 = np.zeros((W, K), dtype=np.float32)
    g10[np.arange(W), group] = 1.0
    shared["g10"] = _fold(g10).astype(ml_dtypes.bfloat16)

    G5 = np.zeros((16, 5), dtype=np.float32)
    G5[:, :4] = GATE_COEF
    G5[:, 4] = 1.0
    ghat = np.zeros((128, 40), dtype=np.float32)
    for k in range(16):
        for m in range(8):
            ghat[k * 8 + m, np.arange(5) * 8 + m] = G5[k]
    shared["ghat"] = ghat.astype(ml_dtypes.bfloat16)

    in_maps = []
    vids = order_v.reshape(8, 128)
    for c in range(NCORES):
        xc = x[c * BC:(c + 1) * BC]               # [32, 1024]
        xt = np.zeros((IN_DIM, E), dtype=ml_dtypes.bfloat16)
        xt[:, :BC] = xc.T.astype(ml_dtypes.bfloat16)
        m = dict(shared)
        m["xT"] = xt
        # xs[p, i, :] = x batch vector of the value at cell (p, i)
        m["xs"] = np.ascontiguousarray(
            xc[:, vids].transpose(2, 1, 0)).astype(ml_dtypes.bfloat16)
        in_maps.append(m)

    res = run_bass_kernel_spmd(nc, in_maps, core_ids=list(range(NCORES)))

    out = np.empty((BATCH, K), dtype=np.float32)
    for c in range(NCORES):
        out[c * BC:(c + 1) * BC] = res.results[c]["out"].T
    return out



# revision 1
# speedup vs baseline: 1.0172x; 1.0172x over previous
"""DiffLogic network TRN2 kernel: 3 logic layers [B=256, W=64000] + GroupSum.

Sharding: pure data-parallel over batch across 8 cores (B=32/core), no
inter-core communication. Per core: activations h stored in DRAM as
[64000, 128] bf16 rows (256B stride, first 32 cols real = 64B payload).
Gathers a=h[idx_a], b=h[idx_b] via SWDGE dma_gather with elem_size=32
(64B descriptors — 7ns floor instead of 22.8ns for 256B) and signed
int16 indices (base at row 32000). h writes go over HWDGE (SP engine)
to keep the Pool engine free for gather descriptor generation.
Gate = (c1+c3*b)*a + (c0+c2*b) computed on DVE (6 tensor-tensor ops,
kept in the 2x perf mode by storing coefficients as stride-1 pairs);
coefs = softmax(w)@G via ACT exp + PE matmuls contracting the 16-gate
axis on the partition dim. GroupSum via PE one-hot matmul.
"""
import numpy as np
import ml_dtypes

import concourse.bass as bass
import concourse.tile as tile
import concourse.bacc as bacc
import concourse.mybir as mybir
from concourse import ap_utils
from concourse.bass_utils import run_bass_kernel_spmd
from concourse.library_config import mlp

W = 64000
BATCH = 256
NCORES = 8
BC = BATCH // NCORES        # 32 batch rows per core
IN_DIM = 1024
K = 10
TAU = 30.0
NSLOT = W // 128            # 500
E = 128                     # elements per h row (256B stride); [:32] real
CHUNK_SLOTS = 64            # neurons per chunk = 64*128 = 8192
GPN = 8192                  # idxs per dma_gather instruction (needs single_packet=False)
H_BASE = 32000              # gather base row (signed int16 rebase)

GATE_COEF = np.array([
    [0., 0., 0., 0.], [0., 0., 0., 1.], [0., 1., 0., -1.], [0., 1., 0., 0.],
    [0., 0., 1., -1.], [0., 0., 1., 0.], [0., 1., 1., -2.], [0., 1., 1., -1.],
    [1., -1., -1., 1.], [1., -1., -1., 2.], [1., 0., -1., 0.], [1., 0., -1., 1.],
    [1., -1., 0., 0.], [1., -1., 0., 1.], [1., 0., 0., -1.], [1., 0., 0., 0.],
], dtype=np.float32)  # [16, 4]

BF16 = mybir.dt.bfloat16
F32 = mybir.dt.float32
I16 = mybir.dt.int16
IDX_COLS = W // 16  # wrapped idx tensor cols per list

_NC_CACHE = {}


def dma_gather_small(gp, out_ap, in_ap, idxs_ap, num_idxs, elem_size, elem_step):
    """dma_gather without the 256B elem-size assert (non-transpose, DRAM src).
    The 256B restriction only applies to transpose mode in the ucode; the
    source row stride (elem_step) must still be a multiple of 256B."""
    assert idxs_ap.dtype == mybir.dt.int16
    assert in_ap.dtype == out_ap.dtype
    assert ap_utils.ap_is_contiguous(out_ap.ap[1:])
    assert ap_utils.ap_is_contiguous(idxs_ap.ap[1:])
    assert in_ap.ap[-1][1] == out_ap.ap[-1][1] == elem_size
    assert in_ap.ap[0][0] == elem_step
    stride_bytes = elem_step * mybir.dt.size(in_ap.dtype)
    stride_256 = stride_bytes // 256
    assert stride_256 * 256 == stride_bytes and stride_256 < 256
    _in_ap = gp.lower_ap_dma(in_ap, for_custom_bir_dma=True)
    _idxs_ap = gp.lower_ap(idxs_ap)
    _out_ap = gp.lower_ap(out_ap)
    return gp.add_instruction(
        mybir.InstDMAGatherAnt(
            name=gp.bass.get_next_instruction_name(),
            ins=[*_in_ap, _idxs_ap, gp.lower_val_access(gp.to_reg(num_idxs))],
            outs=[_out_ap],
            transpose=False,
            num_idxs=num_idxs,
            elem_size=elem_size,
            stride_bytes_256=stride_256,
            gen_mode=0,
            single_packet=False,
            queue_num=0,
            sbuf_tokens_per_rank=0,
            sbuf_free_dim_per_rank=0,
            sbuf_free_dim_pad_per_rank=0,
            sbuf_byte_offset=0,
        ))


def _chunks():
    """Yield (slot0, nslots) chunks over the 500 slots."""
    s = 0
    while s < NSLOT:
        n = min(CHUNK_SLOTS, NSLOT - s)
        yield s, n
        s += n


def _gathers(nslots):
    """Split a chunk of nslots*128 idxs into per-instruction counts."""
    n = nslots * 128
    out = []
    while n > 0:
        g = min(GPN, n)
        out.append(g)
        n -= g
    return out


S1 = 512        # layer-1 grid slots: 8 affine x-columns + spill tail
NPOS1 = S1 * 128


def build_nc(bounds=None, l0meta=None):
    """l0meta: (cols, tail_ns) — cols = [(s0, ns)] for the 8 affine
    x-columns of layer 1 (a read via broadcast from xs, no a-gather),
    tail_ns = slots of the spill chunk (both lists gathered)."""
    if "nc" in _NC_CACHE:
        return _NC_CACHE["nc"]
    nc = bacc.Bacc("TRN2", target_bir_lowering=False, debug=False,
                   enable_asserts=False, num_devices=NCORES)

    NQ1 = S1 // 8          # 64 PE-coef matmuls for layer 1
    NQ = (NSLOT + 7) // 8  # 63 for layers 2-3

    xT = nc.dram_tensor("xT", [IN_DIM, E], BF16, kind="ExternalInput")
    xs = nc.dram_tensor("xs", [128, 8, 32], BF16, kind="ExternalInput")
    # wf2[(k,m), q, K] = w[neuron(p=K, s=8q+m), k]  (PE-coef layout)
    wf = [nc.dram_tensor(f"wf{l}", [128, NQ1 if l == 0 else NQ, 128], BF16,
                         kind="ExternalInput") for l in range(3)]
    ia = [nc.dram_tensor("ia0", [128, 256], I16, kind="ExternalInput")] + [
        nc.dram_tensor(f"ia{l}", [128, IDX_COLS], I16, kind="ExternalInput")
        for l in (1, 2)]
    ib = [nc.dram_tensor("ib0", [128, NPOS1 // 16], I16, kind="ExternalInput")] + [
        nc.dram_tensor(f"ib{l}", [128, IDX_COLS], I16, kind="ExternalInput")
        for l in (1, 2)]
    g10 = nc.dram_tensor("g10", [128, NSLOT, K], BF16, kind="ExternalInput")
    # ghat[(k,m), 5j+mm] = G[k, j] * (m == mm); j=4 row is ones (softmax denom)
    ghat = nc.dram_tensor("ghat", [128, 40], BF16, kind="ExternalInput")
    h_dram = [nc.dram_tensor("h0", [NPOS1, E], BF16, kind="Internal"),
              nc.dram_tensor("h1", [W, E], BF16, kind="Internal")]
    out_dram = nc.dram_tensor("out", [K, BC], F32, kind="ExternalOutput")

    with tile.TileContext(nc) as tc:
        with (
            tc.tile_pool(name="persist", bufs=1) as persist,
            tc.tile_pool(name="idxp", bufs=2) as idxp,
            tc.tile_pool(name="coef", bufs=1) as coefp,
            tc.tile_pool(name="gath", bufs=3) as gath,
            tc.tile_pool(name="temps", bufs=4) as temps,
            tc.tile_pool(name="psum", bufs=1, space="PSUM") as psump,
        ):
            nc.gpsimd.load_library(mlp)

            # persistent loads
            ghat_sb = persist.tile([128, 40], BF16, tag="ghat")
            nc.sync.dma_start(ghat_sb[:], ghat[:])
            g10_sb = persist.tile([128, NSLOT, K], BF16, tag="g10")
            nc.sync.dma_start(g10_sb[:], g10[:])

            psum_out = psump.tile([K, BC], F32, tag="acc")
            n_mm = NSLOT  # total groupsum matmuls
            mm_i = 0

            xs_sb = persist.tile([128, 8, 32], BF16, tag="xs")
            nc.sync.dma_start(xs_sb[:], xs[:])

            for l in range(3):
                NQl = NQ1 if l == 0 else NQ
                Sl = S1 if l == 0 else NSLOT
                if l == 0:
                    ia_sb = idxp.tile([128, 256], I16, tag="iat", name="ia_sb")
                    ib_sb = idxp.tile([128, NPOS1 // 16], I16, tag="ib0t",
                                      name="ib_sb")
                else:
                    ia_sb = idxp.tile([128, IDX_COLS], I16, tag="ia", name="ia_sb")
                    ib_sb = idxp.tile([128, IDX_COLS], I16, tag="ib", name="ib_sb")
                nc.sync.dma_start(ia_sb[:], ia[l][:])
                nc.sync.dma_start(ib_sb[:], ib[l][:])

                # ---- coefficient prep: coef = softmax(w) @ GATE_COEF via PE ----
                # craw[j, n] for neurons n=(K, s=8q+m): matmul q contracts over
                # the (gate k, m) partition dim: out[K, j*8+mm] = sum_p
                # e_fold[p, K] * ghat[p, j*8+mm].
                wf_sb = coefp.tile([128, NQ1, 128], BF16, tag="wf", name="wf_sb")
                nc.sync.dma_start(wf_sb[:, :NQl, :], wf[l][:])
                e_sb = coefp.tile([128, NQ1, 128], BF16, tag="e", name="e_sb")
                nc.scalar.activation(e_sb[:, :NQl, :], wf_sb[:, :NQl, :],
                                     mybir.ActivationFunctionType.Exp)
                c_sb = coefp.tile([128, NQ1, 40], BF16, tag="csb", name="c_sb")
                QG = 12  # matmuls per PSUM bank group
                q0 = 0
                while q0 < NQl:
                    nq = min(QG, NQl - q0)
                    cps = psump.tile([128, QG, 40], F32, tag="cps", name="cps")
                    for qi in range(nq):
                        nc.tensor.matmul(cps[:, qi, :],
                                         lhsT=e_sb[:, q0 + qi, :],
                                         rhs=ghat_sb[:],
                                         start=True, stop=True)
                    nc.scalar.mul(c_sb[:, q0:q0 + nq, :], cps[:, :nq, :], 1.0)
                    q0 += nq
                rinv = coefp.tile([128, NQ1, 8], F32, tag="rinv", name="rinv")
                nc.vector.reciprocal(out=rinv[:, :NQl, :], in_=c_sb[:, :NQl, 32:40])
                # cjd[j][p, s, 0:2] = coef_j[p, s] twice (stride-1 pair so the
                # gate ops' broadcast operand keeps the DVE 2x perf mode)
                cjd = [coefp.tile([128, NQ1 * 8, 2], BF16, tag=f"c{j}",
                                  name=f"cjd{j}") for j in range(4)]
                for j in range(4):
                    dst = (cjd[j][:, :NQl * 8, :]
                           .rearrange("p (q m) t -> p q m t", m=8))
                    cj_s = (c_sb[:, :NQl, j * 8:(j + 1) * 8].unsqueeze(-1)
                            .to_broadcast([128, NQl, 8, 2]))
                    ri_s = (rinv[:, :NQl, :].unsqueeze(-1)
                            .to_broadcast([128, NQl, 8, 2]))
                    nc.vector.tensor_mul(dst, cj_s, ri_s)

                # ---- gather + gate over chunks ----
                def srcs(gi):
                    """Per-instruction source windows: base from the per-
                    instruction int16 rebase, upper bound = max row touched
                    (consumers are sorted by source row, so early gathers
                    depend only on early h writes and can overlap the
                    previous layer's tail)."""
                    if l == 0:
                        return xT[:, :32], xT[:, :32]
                    assert bounds is not None, "layers 2-3 need rebase info"
                    ba, bda, bb, bdb = bounds[l - 1]
                    return (h_dram[l - 1][ba[gi]:bda[gi], :32],
                            h_dram[l - 1][bb[gi]:bdb[gi], :32])

                if l == 0:
                    cols, tail_ns = l0meta
                    chunks = [(s0, ns, i) for i, (s0, ns) in enumerate(cols)]
                    chunks.append((S1 - tail_ns, tail_ns, None))
                else:
                    chunks = [(s0, ns, None) for s0, ns in _chunks()]

                gi = 0
                acol = 0  # a-idx cols consumed (layer-0 tail only)
                for s0, ns, xcol in chunks:
                    a_t = gath.tile([128, CHUNK_SLOTS, 32], BF16, tag="a")
                    b_t = gath.tile([128, CHUNK_SLOTS, 32], BF16, tag="b")
                    col = s0 * 8  # b-idx cols consumed so far (128/16 per slot)
                    slot = 0
                    for n in _gathers(ns):
                        ncols = n // 16
                        nslots_g = n // 128
                        src_a, src_b = srcs(gi)
                        gi += 1
                        if xcol is None:
                            dma_gather_small(
                                nc.gpsimd, a_t[:, slot:slot + nslots_g, :],
                                src_a, ia_sb[:, acol:acol + ncols], n, 32, E)
                            acol += ncols
                        dma_gather_small(
                            nc.gpsimd, b_t[:, slot:slot + nslots_g, :], src_b,
                            ib_sb[:, col:col + ncols], n, 32, E)
                        col += ncols
                        slot += nslots_g

                    # 4-dim views with stride-1 inner pairs keep DVE 2x mode
                    if xcol is None:
                        av = a_t[:, :ns, :].rearrange("p c (g t) -> p c g t", t=2)
                    else:
                        av = (xs_sb[:, xcol, :].rearrange("p (g t) -> p g t", t=2)
                              .unsqueeze(1).to_broadcast([128, ns, 16, 2]))
                    bv = b_t[:, :ns, :].rearrange("p c (g t) -> p c g t", t=2)

                    def cbc(j):
                        return (cjd[j][:, s0:s0 + ns, :].unsqueeze(2)
                                .to_broadcast([128, ns, 16, 2]))

                    # gate = (c1 + c3*b)*a + (c0 + c2*b): 6 DVE ops
                    m1 = temps.tile([128, CHUNK_SLOTS, 32], BF16, tag="m1")
                    m2 = temps.tile([128, CHUNK_SLOTS, 32], BF16, tag="m2")
                    m1v = m1[:, :ns, :].rearrange("p c (g t) -> p c g t", t=2)
                    m2v = m2[:, :ns, :].rearrange("p c (g t) -> p c g t", t=2)
                    nc.vector.tensor_mul(m1v, bv, cbc(3))
                    nc.vector.tensor_add(m1v, m1v, cbc(1))
                    nc.vector.tensor_mul(m1v, m1v, av)
                    nc.vector.tensor_mul(m2v, bv, cbc(2))
                    nc.vector.tensor_add(m2v, m2v, cbc(0))
                    nc.vector.tensor_add(m1v, m1v, m2v)

                    if l < 2:
                        # write rows (s0+c)*128+p of h_dram[l] (64B @ 256B stride)
                        hap = h_dram[l].ap()
                        dst = hap[s0 * 128: s0 * 128 + ns * 128, :32]
                        dst = dst.rearrange("(c p) e -> p c e", p=128)
                        nc.sync.dma_start(dst, m1[:, :ns, :])
                    else:
                        for c in range(ns):
                            nc.tensor.matmul(
                                psum_out[:],
                                lhsT=g10_sb[:, s0 + c, :],
                                rhs=m1[:, c, :],
                                start=(mm_i == 0),
                                stop=(mm_i == n_mm - 1),
                            )
                            mm_i += 1

            out_sb = persist.tile([K, BC], F32, tag="outsb")
            nc.scalar.mul(out_sb[:], psum_out[:], 1.0 / TAU)
            nc.sync.dma_start(out_dram[:], out_sb[:])

    nc.compile()
    _NC_CACHE["nc"] = nc
    return nc


def _wrap(idx):
    """Flat idx list [n] -> [128, n/16] int16 wrapped per 16 partitions,
    replicated to the 8 gpsimd cores."""
    n = idx.shape[0]
    arr = np.empty((128, n // 16), dtype=np.int16)
    blk = idx.reshape(n // 16, 16).T.astype(np.int16)
    for g in range(8):
        arr[g * 16:(g + 1) * 16, :] = blk
    return arr


def _fix_trailing(idx_a, idx_b, perm=None):
    """Ensure the last idx of every GPN-sublist is >= 0 for both lists
    (SWDGE trims trailing negatives). Returns permuted lists + perm."""
    perm = np.arange(W) if perm is None else perm.copy()
    a = idx_a.copy()
    b = idx_b.copy()
    pos = 0
    for s0, ns in _chunks():
        for n in _gathers(ns):
            last = pos + n - 1
            if a[last] < 0 or b[last] < 0:
                ok = np.nonzero((a[pos:last] >= 0) & (b[pos:last] >= 0))[0]
                j = pos + int(ok[-1])
                for arr in (a, b, perm):
                    arr[last], arr[j] = arr[j], arr[last]
            pos += n
    return a, b, perm


def _fold(x):
    """[W, ...] -> [128, NSLOT, ...] with row n=(c*128+p) at [p, c]."""
    return np.ascontiguousarray(
        x.reshape(NSLOT, 128, *x.shape[1:]).transpose(1, 0, *range(2, x.ndim + 1)))


def kernel(x, w1, w2, w3, idx_a1, idx_b1, idx_a2, idx_b2, idx_a3, idx_b3):
    x = np.asarray(x, dtype=np.float32)
    ws = [np.asarray(w, dtype=np.float32) for w in (w1, w2, w3)]
    ias = [np.asarray(i).astype(np.int64) for i in (idx_a1, idx_a2, idx_a3)]
    ibs = [np.asarray(i).astype(np.int64) for i in (idx_b1, idx_b2, idx_b3)]

    # ---- host-side index translation / layout prep (shared across cores) ----
    # Layer 1: assign x value of count-rank r to cell (p=r%128, col=r//128);
    # consumers sit in their a-source's cell (up to the column cap), so the
    # a-operand is an affine broadcast from xs and needs NO gather. Excess
    # consumers spill to a tail chunk where both lists are gathered.
    T_CAP = 64
    counts = np.bincount(ias[0], minlength=IN_DIM)
    order_v = np.argsort(-counts, kind="stable")
    Mcol = np.minimum(counts[order_v].reshape(8, 128).max(axis=1),
                      T_CAP).astype(np.int64)
    offs = np.concatenate([[0], np.cumsum(Mcol)])
    S_AFF = int(Mcol.sum())
    TAIL = S1 - S_AFF
    cols = [(int(offs[i]), int(Mcol[i])) for i in range(8)]
    ordc = np.argsort(ias[0], kind="stable")
    starts = np.concatenate([[0], np.cumsum(counts)])
    perm0 = np.full(NPOS1, -1, dtype=np.int64)
    tail_cons = []
    for r in range(IN_DIM):
        v = int(order_v[r])
        p, i = r % 128, r // 128
        cons = ordc[starts[v]:starts[v + 1]]
        k = min(len(cons), int(Mcol[i]))
        perm0[(offs[i] + np.arange(k)) * 128 + p] = cons[:k]
        tail_cons.extend(cons[k:].tolist())
    assert len(tail_cons) <= TAIL * 128
    perm0[S_AFF * 128 + np.arange(len(tail_cons))] = tail_cons
    mask0 = perm0 >= 0
    b1 = np.zeros(NPOS1, dtype=np.int64)
    b1[mask0] = ibs[0][perm0[mask0]]
    a_tail = np.zeros(TAIL * 128, dtype=np.int64)
    tmask = mask0[S_AFF * 128:]
    a_tail[tmask] = ias[0][perm0[S_AFF * 128:][tmask]]
    a_tail_pad = np.zeros(256 * 16, dtype=np.int64)
    a_tail_pad[:TAIL * 128] = a_tail

    perms = [perm0]
    lists = [None]
    bounds = []
    for l in (1, 2):
        pm = perms[l - 1]
        inv_prev = np.empty(W, dtype=np.int64)
        msk = pm >= 0
        inv_prev[pm[msk]] = np.nonzero(msk)[0]
        ra = inv_prev[ias[l]]
        rb = inv_prev[ibs[l]]
        # sort consumers by max source row: early gather instructions then
        # only touch early h rows, so (with per-instruction src-AP windows)
        # they can start before the previous layer finishes writing h
        order = np.argsort(np.maximum(ra, rb), kind="stable")
        ra2, rb2 = ra[order].copy(), rb[order].copy()
        # per-instruction int16 rebase: base = max(0, hi-32767) keeps every
        # idx in range; sorted sublists have narrow spans so most have no
        # negative idxs at all
        binfo = ([], [], [], [])  # base_a, bound_a, base_b, bound_b
        pos = 0
        for s0, ns in _chunks():
            for n in _gathers(ns):
                sl = slice(pos, pos + n)
                for arr, k in ((ra2, 0), (rb2, 2)):
                    hi = int(arr[sl].max())
                    base = max(0, hi - 32767)
                    arr[sl] -= base
                    binfo[k].append(base)
                    binfo[k + 1].append(hi + 1)
                pos += n
        ra3, rb3, perm = _fix_trailing(ra2, rb2, perm=order)
        perms.append(perm)
        lists.append((ra3, rb3))
        bounds.append(binfo)

    nc = build_nc(bounds, l0meta=(cols, TAIL))

    NQ = (NSLOT + 7) // 8
    NQ1 = S1 // 8

    def _wf_fold(wp, S, NQl):
        # wf2[k*8+m, q, K] = wp[(8q+m)*128 + K, k]
        wf2 = np.zeros((128, NQl, 128), dtype=np.float32)
        for m in range(8):
            s_ids = 8 * np.arange(NQl) + m
            valid = s_ids < S
            n = s_ids[valid][:, None] * 128 + np.arange(128)[None, :]
            vals = wp[n, :].transpose(2, 0, 1)    # [16, nq_v, 128]
            tmp = np.zeros((16, NQl, 128), dtype=np.float32)
            tmp[:, valid, :] = vals
            wf2[np.arange(16) * 8 + m] = tmp
        return wf2.astype(ml_dtypes.bfloat16)

    shared = {}
    shared["ia0"] = _wrap(a_tail_pad)
    shared["ib0"] = _wrap(b1)
    wp0 = np.zeros((NPOS1, 16), dtype=np.float32)
    wp0[mask0] = ws[0][perm0[mask0]]
    shared["wf0"] = _wf_fold(wp0, S1, NQ1)
    for l in (1, 2):
        a, b = lists[l]
        shared[f"ia{l}"] = _wrap(a)
        shared[f"ib{l}"] = _wrap(b)
        shared[f"wf{l}"] = _wf_fold(ws[l][perms[l]], NSLOT, NQ)

    group = perms[2] // (W // K)          # group id of neuron at list pos j
    g10 = np.zeros((W, K), dtype=np.float32)
    g10[np.arange(W), group] = 1.0
    shared["g10"] = _fold(g10).astype(ml_dtypes.bfloat16)

    G5 = np.zeros((16, 5), dtype=np.float32)
    G5[:, :4] = GATE_COEF
    G5[:, 4] = 1.0
    ghat = np.zeros((128, 40), dtype=np.float32)
    for k in range(16):
        for m in range(8):
            ghat[k * 8 + m, np.arange(5) * 8 + m] = G5[k]
    shared["ghat"] = ghat.astype(ml_dtypes.bfloat16)

    in_maps = []
    vids = order_v.reshape(8, 128)
    for c in range(NCORES):
        xc = x[c * BC:(c + 1) * BC]               # [32, 1024]
        xt = np.zeros((IN_DIM, E), dtype=ml_dtypes.bfloat16)
        xt[:, :BC] = xc.T.astype(ml_dtypes.bfloat16)
        m = dict(shared)
        m["xT"] = xt
        # xs[p, i, :] = x batch vector of the value at cell (p, i)
        m["xs"] = np.ascontiguousarray(
            xc[:, vids].transpose(2, 1, 0)).astype(ml_dtypes.bfloat16)
        in_maps.append(m)

    res = run_bass_kernel_spmd(nc, in_maps, core_ids=list(range(NCORES)))

    out = np.empty((BATCH, K), dtype=np.float32)
    for c in range(NCORES):
        out[c * BC:(c + 1) * BC] = res.results[c]["out"].T
    return out

